# revision 13
# baseline (speedup 1.0000x reference)
"""Trainium2 Bass kernel for the DQN-GNN (2-layer NNConv + DQN heads).

Strategy (8 NeuronCores, SPMD single program):
  - Host: sort edges by dst, shard edge-parallel by dst-ownership (1250
    nodes/core), pad each 128-node window to a uniform tile count, build
    inv-degree-scaled one-hot scatter matrices, int16 gather indices.
  - Device per core: edge-MLP via PE matmuls (aT computed transposed),
    per-edge matvec via DVE broadcast-multiply + strided reduce,
    scatter-mean via one-hot matmul accumulation in PSUM, node update
    (root/bias folded into same PSUM group), relu+LN, AllGather h,
    dma_gather for h[src]; same again for layer 2; DQN heads with PE
    transposes + fused scalar_tensor_tensor reductions.
  - Host: unshard pipe Q-values, concat with q_global.
"""

import numpy as np

import concourse.bass as bass
import concourse.mybir as mybir
import concourse.bacc as bacc
from concourse import tile
from concourse.bass_utils import run_bass_kernel_spmd

F32 = mybir.dt.float32
I16 = mybir.dt.int16
AX = mybir.AxisListType
OP = mybir.AluOpType
ACTF = mybir.ActivationFunctionType

# problem constants (hardcoded per harness contract)
N, E, P = 10000, 50000, 20000
NODE_IN, EDGE_IN, H = 4, 4, 64
EPS = 1e-5
NCORES = 8
NPC = N // NCORES          # 1250 nodes owned per core
NW = 10                    # 128-node windows per core
NPAD = NW * 128            # 1280 padded node slots per core
PPC = P // NCORES          # 2500 pipes per core
PT = (PPC + 127) // 128    # 20 pipe tiles
PPAD = PT * 128

LAST_RESULT = None
BUILD_STAGE = 99


def _pack_idx16(idx, total):
    """Pack indices into the (16, total//16) SWDGE layout: elem i at [i%16, i//16]."""
    flat = np.zeros(total, np.int64)
    flat[: len(idx)] = idx
    blk = flat.reshape(total // 16, 16).T.astype(np.int16)
    return np.ascontiguousarray(np.tile(blk, (8, 1)))


def _table_row(n):
    """Node id -> row in the all-gathered (NCORES*NPAD, H) h table."""
    return (n // NPC) * NPAD + (n % NPC)


def _host_prep(x, edge_index, edge_attr, pipe_edge_idx, pipe_open_mask):
    src = edge_index[0].astype(np.int64)
    dst = edge_index[1].astype(np.int64)
    deg = np.bincount(dst, minlength=N).astype(np.float32)
    inv_cnt = (1.0 / np.maximum(deg, 1.0)).astype(np.float32)

    order = np.argsort(dst, kind="stable")

    # per (core, window) edge lists (edges sorted by dst already)
    per_cw = [[None] * NW for _ in range(NCORES)]
    dst_s = dst[order]
    core_s = dst_s // NPC
    win_s = (dst_s - core_s * NPC) // 128
    for c in range(NCORES):
        mask_c = core_s == c
        for w in range(NW):
            per_cw[c][w] = order[mask_c & (win_s == w)]
    T_w = 1
    for c in range(NCORES):
        for w in range(NW):
            T_w = max(T_w, -(-len(per_cw[c][w]) // 128))
    NT = NW * T_w
    E_pad = NT * 128

    cores = []
    for c in range(NCORES):
        perm = np.full((NT, 128), -1, np.int64)
        for w in range(NW):
            lst = per_cw[c][w]
            for s, e in enumerate(lst):
                perm[w * T_w + s // 128, s % 128] = e
        flat = perm.reshape(-1)
        valid = flat >= 0
        fsafe = np.where(valid, flat, 0)

        onehot = np.zeros((NT, 128, 128), np.float32)
        t_idx, p_idx = np.nonzero(perm >= 0)
        e_ids = perm[t_idx, p_idx]
        loc = dst[e_ids] - c * NPC - (t_idx // T_w) * 128
        onehot[t_idx, p_idx, loc] = inv_cnt[dst[e_ids]]

        eaT = np.ascontiguousarray(
            np.where(valid[:, None], edge_attr[fsafe], 0.0).T.astype(np.float32))
        xsrc0 = np.where(valid[:, None], x[src[fsafe]], 0.0).astype(np.float32)
        xsrc0 = np.ascontiguousarray(
            xsrc0.reshape(NT, 128, NODE_IN).transpose(1, 0, 2))  # (128, NT, 4)
        gidx = _pack_idx16(np.where(valid, _table_row(src[fsafe]), 0), E_pad)

        xT = np.zeros((NODE_IN, NPAD), np.float32)
        xT[:, :NPC] = x[c * NPC:(c + 1) * NPC].T

        pids = np.arange(c * PPC, (c + 1) * PPC)
        pe = pipe_edge_idx[pids].astype(np.int64)
        gsu = _pack_idx16(_table_row(src[pe]), PPAD)
        gsv = _pack_idx16(_table_row(dst[pe]), PPAD)
        m = np.zeros((PPAD, 2), np.float32)
        m[:PPC, 0] = pipe_open_mask[pids]
        m[:PPC, 1] = 1.0 - pipe_open_mask[pids]
        cb = np.where(m > 0.5, np.float32(0.0), np.float32(-1e9))
        cb[PPC:] = 0.0
        m_t = np.ascontiguousarray(
            m.reshape(PT, 128, 2).transpose(1, 0, 2).reshape(128, PT * 2))
        cb_t = np.ascontiguousarray(
            cb.reshape(PT, 128, 2).transpose(1, 0, 2).reshape(128, PT * 2))

        cores.append(dict(
            onehot=onehot, eaT=eaT, xsrc0=xsrc0, gidx=gidx, xT=xT,
            gsu=gsu, gsv=gsv, m=m_t, cb=cb_t,
        ))
    return cores, T_w, NT, E_pad


def _bcast(v, parts=128):
    v = np.asarray(v, np.float32).reshape(1, -1)
    return np.ascontiguousarray(np.broadcast_to(v, (parts, v.shape[1])))


def _prep_params(params):
    c0, c1 = params["convs"][0], params["convs"][1]
    qn_w = np.asarray(params["qn_w"], np.float32)
    pp = dict(
        e1w0=np.asarray(c0["e1_w"], np.float32),
        e1b0=np.asarray(c0["e1_b"], np.float32).reshape(1, -1),
        e2w0=np.asarray(c0["e2_w"], np.float32),
        e2b0=np.asarray(c0["e2_b"], np.float32).reshape(1, -1),
        root0=np.asarray(c0["root"], np.float32),
        bias0=np.asarray(c0["bias"], np.float32).reshape(1, -1),
        ln0g=_bcast(c0["ln_g"]), ln0b=_bcast(c0["ln_b"]),
        e1w1=np.asarray(c1["e1_w"], np.float32),
        e1b1=np.asarray(c1["e1_b"], np.float32).reshape(1, -1),
        e2w1=np.asarray(c1["e2_w"], np.float32),
        e2b1=np.asarray(c1["e2_b"], np.float32).reshape(1, -1),
        root1=np.asarray(c1["root"], np.float32),
        bias1=np.asarray(c1["bias"], np.float32).reshape(1, -1),
        ln1g=_bcast(c1["ln_g"]), ln1b=_bcast(c1["ln_b"]),
        pairw=np.asarray(params["pair_w"], np.float32),
        pairb=_bcast(params["pair_b"]),
        plng=_bcast(params["pair_ln_g"]), plnb=_bcast(params["pair_ln_b"]),
        qnw=np.concatenate([_bcast(qn_w[:, j]) for j in range(2)], axis=1),
        qnb=np.asarray(params["qn_b"], np.float32).reshape(1, 2),
        ghw=np.asarray(params["gh_w"], np.float32),
        ghb=np.asarray(params["gh_b"], np.float32).reshape(1, -1),
        glng=np.asarray(params["gh_ln_g"], np.float32).reshape(1, -1),
        glnb=np.asarray(params["gh_ln_b"], np.float32).reshape(1, -1),
        qgw=np.ascontiguousarray(np.asarray(params["qg_w"], np.float32).T.reshape(1, 128)),
        qgb=np.asarray(params["qg_b"], np.float32).reshape(1, 2),
        ident=np.eye(128, dtype=np.float32),
    )
    return pp


PARAM_SHAPES = dict(
    e1w0=(4, 64), e1b0=(1, 64), e2w0=(64, 256), e2b0=(1, 256), root0=(4, 64),
    bias0=(1, 64), ln0g=(128, 64), ln0b=(128, 64),
    e1w1=(4, 128), e1b1=(1, 128), e2w1=(128, 4096), e2b1=(1, 4096), root1=(64, 64),
    bias1=(1, 64), ln1g=(128, 64), ln1b=(128, 64),
    pairw=(128, 64), pairb=(128, 64), plng=(128, 64), plnb=(128, 64),
    qnw=(128, 128), qnb=(1, 2),
    ghw=(64, 64), ghb=(1, 64), glng=(1, 64), glnb=(1, 64),
    qgw=(1, 128), qgb=(1, 2), ident=(128, 128),
)


def _emit_layernorm(nc, sb, x_ap, g_ap, b_ap, out_ap, parts, width, eps_ap):
    """out = (x - mean)/sqrt(var + EPS) * g + b, per-partition over free dim."""
    s = sb.tile([parts, 1], F32, tag="ln_s")
    mu = sb.tile([parts, 1], F32, tag="ln_mu")
    cen = sb.tile([parts, width], F32, tag="ln_cen")
    var = sb.tile([parts, 1], F32, tag="ln_var")
    junk = sb.tile([parts, width], F32, tag="ln_junk")
    std = sb.tile([parts, 1], F32, tag="ln_std")
    rstd = sb.tile([parts, 1], F32, tag="ln_rstd")
    nc.vector.reduce_sum(s[:], x_ap, axis=AX.X)
    nc.vector.tensor_scalar_mul(mu[:], s[:], 1.0 / width)
    nc.vector.tensor_scalar_sub(cen[:], x_ap, mu[:])
    nc.vector.tensor_tensor(junk[:], cen[:], cen[:], OP.mult)
    nc.vector.reduce_sum(var[:], junk[:], axis=AX.X)
    nc.scalar.activation(std[:], var[:], ACTF.Sqrt, bias=eps_ap[:parts, :],
                         scale=1.0 / width)
    nc.vector.reciprocal(rstd[:], std[:])
    nc.vector.scalar_tensor_tensor(out_ap, cen[:], rstd[:], g_ap, OP.mult, OP.mult)
    nc.vector.tensor_tensor(out_ap, out_ap, b_ap, OP.add)


class _Ctx:
    pass


class _StopBuild(Exception):
    pass


def build_program(T_w, NT, E_pad, pp):
    nc = bacc.Bacc("TRN2", target_bir_lowering=False, debug=False,
                   num_devices=NCORES)
    RG = [list(range(NCORES))]

    has_e1b0 = bool(np.any(pp["e1b0"] != 0))
    has_e1b1 = bool(np.any(pp["e1b1"] != 0))
    has_e2b0 = bool(np.any(pp["e2b0"] != 0))
    has_e2b1 = bool(np.any(pp["e2b1"] != 0))

    din = {}
    for name, shp in PARAM_SHAPES.items():
        din[name] = nc.dram_tensor(name, shp, F32, kind="ExternalInput")
    din["onehot"] = nc.dram_tensor("onehot", (NT, 128, 128), F32, kind="ExternalInput")
    din["eaT"] = nc.dram_tensor("eaT", (4, E_pad), F32, kind="ExternalInput")
    din["xsrc0"] = nc.dram_tensor("xsrc0", (128, NT, NODE_IN), F32, kind="ExternalInput")
    din["gidx"] = nc.dram_tensor("gidx", (128, E_pad // 16), I16, kind="ExternalInput")
    din["xT"] = nc.dram_tensor("xT", (NODE_IN, NPAD), F32, kind="ExternalInput")
    din["gsu"] = nc.dram_tensor("gsu", (128, PPAD // 16), I16, kind="ExternalInput")
    din["gsv"] = nc.dram_tensor("gsv", (128, PPAD // 16), I16, kind="ExternalInput")
    din["m"] = nc.dram_tensor("m", (128, PT * 2), F32, kind="ExternalInput")
    din["cb"] = nc.dram_tensor("cb", (128, PT * 2), F32, kind="ExternalInput")

    out_q = nc.dram_tensor("out_q", (128, PT * 2), F32, kind="ExternalOutput")
    out_qg = nc.dram_tensor("out_qg", (1, 2), F32, kind="ExternalOutput")

    h1chunk = nc.dram_tensor("h1chunk", (NPAD, H), F32, kind="Internal")
    h1full = nc.dram_tensor("h1full", (NCORES * NPAD, H), F32, kind="Internal",
                            addr_space="Shared")
    h2chunk = nc.dram_tensor("h2chunk", (NPAD, H), F32, kind="Internal")
    h2full = nc.dram_tensor("h2full", (NCORES * NPAD, H), F32, kind="Internal",
                            addr_space="Shared")
    gin = nc.dram_tensor("gin", (1, H), F32, kind="Internal")
    gout = nc.dram_tensor("gout", (1, H), F32, kind="Internal", addr_space="Shared")

    NG = -(-NT // 4)  # 512-edge groups

    with tile.TileContext(nc) as tc:
      try:
        with (
            tc.tile_pool(name="const", bufs=1) as cp,
            tc.tile_pool(name="sb", bufs=2) as sb,
            tc.tile_pool(name="lnp", bufs=2) as lnp,
            tc.tile_pool(name="hold", bufs=NW + 1) as hold,
            tc.tile_pool(name="msgs", bufs=T_w + 4) as msgp,
            tc.tile_pool(name="psA", bufs=1, space="PSUM") as psA,
            tc.tile_pool(name="psW", bufs=2, space="PSUM") as psW,
            tc.tile_pool(name="psG", bufs=1, space="PSUM") as psG,
            tc.tile_pool(name="psT", bufs=2, space="PSUM") as psT,
        ):
            X = _Ctx()
            X.nc, X.sb, X.lnp, X.hold = nc, sb, lnp, hold
            X.psG, X.psT, X.din = psG, psT, din
            X.T_w = T_w

            pt = {}
            for name, shp in PARAM_SHAPES.items():
                pt[name] = cp.tile(list(shp), F32, tag=f"c_{name}", name=f"ct_{name}")
                nc.sync.dma_start(pt[name][:], din[name][:])
            X.pt = pt
            ones_row = cp.tile([1, 512], F32, tag="ones_row")
            nc.vector.memset(ones_row[:], 1.0)
            X.ones = ones_row
            eps_t = cp.tile([128, 1], F32, tag="eps_t")
            nc.vector.memset(eps_t[:], EPS)
            X.eps = eps_t

            xsrc0 = cp.tile([128, NT, NODE_IN], F32, tag="xsrc0")
            nc.sync.dma_start(xsrc0[:], din["xsrc0"][:])
            gidx = cp.tile([128, E_pad // 16], I16, tag="gidx")
            nc.sync.dma_start(gidx[:], din["gidx"][:])
            xTt = cp.tile([NODE_IN, NPAD], F32, tag="xT")
            nc.sync.dma_start(xTt[:], din["xT"][:])
            X.xT = xTt

            X.h1T_tiles = []
            X.gacc = cp.tile([1, NW * H], F32, tag="gacc")

            # ================= layer 0 (conv0) =================
            msg_tiles = {}
            next_w = 0
            for g in range(NG):
                gt = min(4, NT - 4 * g)
                ncols = gt * 128
                ea_g = sb.tile([4, 512], F32, tag="ea_g")
                nc.sync.dma_start(ea_g[:, :ncols], din["eaT"][:, g * 512: g * 512 + ncols])
                aT0_ps = psA.tile([128, 512], F32, tag="aT_ps")
                nc.tensor.matmul(aT0_ps[:64, :ncols], pt["e1w0"][:], ea_g[:, :ncols],
                                 start=True, stop=not has_e1b0)
                if has_e1b0:
                    nc.tensor.matmul(aT0_ps[:64, :ncols], pt["e1b0"][:],
                                     ones_row[:, :ncols], start=False, stop=True)
                aT0 = sb.tile([64, 512], F32, tag="aT0_sb")
                nc.scalar.activation(aT0[:, :ncols], aT0_ps[:64, :ncols], ACTF.Relu)

                for ti in range(gt):
                    t = 4 * g + ti
                    w0_ps = psW.tile([128, 1024], F32, tag="wq")
                    nc.tensor.matmul(w0_ps[:, :256], aT0[:, ti * 128:(ti + 1) * 128],
                                     pt["e2w0"][:], start=True, stop=not has_e2b0)
                    if has_e2b0:
                        nc.tensor.matmul(w0_ps[:, :256], ones_row[:, :128],
                                         pt["e2b0"][:], start=False, stop=True)
                    y0 = sb.tile([128, 1024], F32, tag="y")
                    y0v = y0[:, :256].rearrange("p (i o) -> p i o", o=H)
                    w0v = w0_ps[:, :256].rearrange("p (i o) -> p i o", o=H)
                    xv = xsrc0[:, t, :].unsqueeze(2).broadcast_to((128, NODE_IN, H))
                    nc.vector.tensor_tensor(y0v, w0v, xv, OP.mult)
                    msg = msgp.tile([128, H], F32, tag="msg")
                    nc.vector.reduce_sum(
                        msg[:], y0[:, :256].rearrange("p (i o) -> p o i", o=H),
                        axis=AX.X)
                    msg_tiles[t] = msg

                while next_w < NW and 4 * g + gt >= (next_w + 1) * T_w:
                    _emit_window(X, msg_tiles, next_w, layer=0, chunk=h1chunk)
                    next_w += 1

            # ---- AllGather h1, gather h1[src] ----
            if BUILD_STAGE < 2:
                raise _StopBuild()
            nc.gpsimd.collective_compute(
                "AllGather", OP.bypass, replica_groups=RG,
                ins=[h1chunk[:]], outs=[h1full[:]])
            hs1 = cp.tile([128, NT, H], F32, tag="hs1")
            for b in range(0, E_pad, 1024):
                nn = min(1024, E_pad - b)
                nc.gpsimd.dma_gather(
                    hs1[:, b // 128:(b + nn) // 128, :], h1full[:],
                    gidx[:, b // 16:(b + nn) // 16], nn, nn, H)

            # ================= layer 1 (conv1) =================
            if BUILD_STAGE < 3:
                raise _StopBuild()
            msg_tiles1 = {}
            next_w = 0
            for g in range(NG):
                gt = min(4, NT - 4 * g)
                ncols = gt * 128
                ea_g = sb.tile([4, 512], F32, tag="ea_g")
                nc.sync.dma_start(ea_g[:, :ncols], din["eaT"][:, g * 512: g * 512 + ncols])
                aT1_ps = psA.tile([128, 512], F32, tag="aT_ps")
                nc.tensor.matmul(aT1_ps[:, :ncols], pt["e1w1"][:], ea_g[:, :ncols],
                                 start=True, stop=not has_e1b1)
                if has_e1b1:
                    nc.tensor.matmul(aT1_ps[:, :ncols], pt["e1b1"][:],
                                     ones_row[:, :ncols], start=False, stop=True)
                aT1 = sb.tile([128, 512], F32, tag="aT1_sb")
                nc.scalar.activation(aT1[:, :ncols], aT1_ps[:, :ncols], ACTF.Relu)

                for ti in range(gt):
                    t = 4 * g + ti
                    msgq = sb.tile([128, H, 4], F32, tag="msgq")
                    for q in range(4):
                        w1q = psW.tile([128, 1024], F32, tag="wq")
                        for hf in range(2):
                            lo = q * 1024 + hf * 512
                            nc.tensor.matmul(
                                w1q[:, hf * 512:(hf + 1) * 512],
                                aT1[:, ti * 128:(ti + 1) * 128],
                                pt["e2w1"][:, lo: lo + 512],
                                start=True, stop=not has_e2b1)
                            if has_e2b1:
                                nc.tensor.matmul(
                                    w1q[:, hf * 512:(hf + 1) * 512],
                                    ones_row[:, :128],
                                    pt["e2b1"][:, lo: lo + 512],
                                    start=False, stop=True)
                        y = sb.tile([128, 1024], F32, tag="y")
                        yv = y[:].rearrange("p (i o) -> p i o", o=H)
                        w1v = w1q[:].rearrange("p (i o) -> p i o", o=H)
                        hv = hs1[:, t, q * 16:(q + 1) * 16].unsqueeze(2).broadcast_to(
                            (128, 16, H))
                        nc.vector.tensor_tensor(yv, w1v, hv, OP.mult)
                        nc.vector.reduce_sum(
                            msgq[:, :, q], y[:].rearrange("p (i o) -> p o i", o=H),
                            axis=AX.X)
                    msg = msgp.tile([128, H], F32, tag="msg")
                    nc.vector.reduce_sum(msg[:], msgq[:], axis=AX.X)
                    msg_tiles1[t] = msg

                while next_w < NW and 4 * g + gt >= (next_w + 1) * T_w:
                    _emit_window(X, msg_tiles1, next_w, layer=1, chunk=h2chunk)
                    next_w += 1

            # ---- AllGather h2 + AllReduce g ----
            if BUILD_STAGE < 4:
                raise _StopBuild()
            nc.gpsimd.collective_compute(
                "AllGather", OP.bypass, replica_groups=RG,
                ins=[h2chunk[:]], outs=[h2full[:]])
            gsum = sb.tile([1, H], F32, tag="gsum")
            nc.vector.reduce_sum(
                gsum[:], X.gacc[:].rearrange("p (w f) -> p f w", f=H), axis=AX.X)
            nc.sync.dma_start(gin[:], gsum[:])
            nc.gpsimd.collective_compute(
                "AllReduce", OP.add, replica_groups=RG,
                ins=[gin[:]], outs=[gout[:]])

            # ---- global head (redundant on every core) ----
            if BUILD_STAGE < 5:
                raise _StopBuild()
            gcol = sb.tile([H, 1], F32, tag="gcol")
            nc.sync.dma_start(gcol[:], gout[:].squeeze(0).unsqueeze(1))
            gcols = sb.tile([H, 1], F32, tag="gcols")
            nc.vector.tensor_scalar_mul(gcols[:], gcol[:], 1.0 / N)
            ghp = psT.tile([128, 128], F32, tag="psT")
            nc.tensor.matmul(ghp[:1, :H], gcols[:], pt["ghw"][:], start=True, stop=False)
            nc.tensor.matmul(ghp[:1, :H], ones_row[:, :1], pt["ghb"][:],
                             start=False, stop=True)
            ghr = sb.tile([1, H], F32, tag="ghr")
            nc.scalar.activation(ghr[:], ghp[:1, :H], ACTF.Relu)
            ghf = sb.tile([1, H], F32, tag="ghf")
            _emit_layernorm(nc, lnp, ghr[:], pt["glng"][:], pt["glnb"][:], ghf[:], 1, H, eps_t)
            qgl = sb.tile([1, 2], F32, tag="qgl")
            junkg = sb.tile([1, H], F32, tag="junkg")
            for j in range(2):
                nc.vector.scalar_tensor_tensor(
                    junkg[:], ghf[:], 1.0, pt["qgw"][:, j * H:(j + 1) * H], OP.mult, OP.mult,
                    accum_out=qgl[:, j:j + 1])
            qglo = sb.tile([1, 2], F32, tag="qglo")
            nc.vector.tensor_tensor(qglo[:], qgl[:], pt["qgb"][:], OP.add)
            nc.sync.dma_start(out_qg[:], qglo[:])
            qadd = sb.tile([1, 2], F32, tag="qadd")
            nc.vector.tensor_tensor(qadd[:], qglo[:], pt["qnb"][:], OP.add)
            qadd_ps = psT.tile([128, 128], F32, tag="psT")
            nc.tensor.matmul(qadd_ps[:, :2], ones_row[:, :128], qadd[:],
                             start=True, stop=True)
            qadd_b = sb.tile([128, 2], F32, tag="qadd_b")
            nc.vector.tensor_copy(qadd_b[:], qadd_ps[:, :2])

            # ---- pipe head ----
            if BUILD_STAGE < 6:
                raise _StopBuild()
            pu = cp.tile([128, PT, H], F32, tag="pu")
            pv = cp.tile([128, PT, H], F32, tag="pv")
            gsu_t = cp.tile([128, PPAD // 16], I16, tag="gsu")
            gsv_t = cp.tile([128, PPAD // 16], I16, tag="gsv")
            nc.sync.dma_start(gsu_t[:], din["gsu"][:])
            nc.sync.dma_start(gsv_t[:], din["gsv"][:])
            for b in range(0, PPAD, 1024):
                nn = min(1024, PPAD - b)
                nc.gpsimd.dma_gather(
                    pu[:, b // 128:(b + nn) // 128, :], h2full[:],
                    gsu_t[:, b // 16:(b + nn) // 16], nn, nn, H)
                nc.gpsimd.dma_gather(
                    pv[:, b // 128:(b + nn) // 128, :], h2full[:],
                    gsv_t[:, b // 16:(b + nn) // 16], nn, nn, H)

            m_t = cp.tile([128, PT * 2], F32, tag="m_t")
            cb_t = cp.tile([128, PT * 2], F32, tag="cb_t")
            nc.sync.dma_start(m_t[:], din["m"][:])
            nc.sync.dma_start(cb_t[:], din["cb"][:])

            qall = cp.tile([128, PT * 2], F32, tag="qall")
            for t in range(PT):
                pairT = sb.tile([128, 128], F32, tag="pairT")
                for half, srct in ((0, pu), (1, pv)):
                    tr_ps = psT.tile([128, 128], F32, tag="psT")
                    nc.tensor.transpose(tr_ps[:H, :], srct[:, t, :], pt["ident"][:])
                    nc.vector.tensor_copy(pairT[half * H:(half + 1) * H, :],
                                          tr_ps[:H, :])
                feat_ps = psT.tile([128, 128], F32, tag="psT")
                nc.tensor.matmul(feat_ps[:, :H], pairT[:], pt["pairw"][:],
                                 start=True, stop=True)
                featb = sb.tile([128, H], F32, tag="featb")
                nc.vector.tensor_tensor(featb[:], feat_ps[:, :H], pt["pairb"][:],
                                        OP.add)
                featr = sb.tile([128, H], F32, tag="featr")
                nc.scalar.activation(featr[:], featb[:], ACTF.Relu)
                featf = sb.tile([128, H], F32, tag="featf")
                _emit_layernorm(nc, lnp, featr[:], pt["plng"][:], pt["plnb"][:],
                                featf[:], 128, H, eps_t)
                junkp = sb.tile([128, H], F32, tag="junkp")
                for j in range(2):
                    nc.vector.scalar_tensor_tensor(
                        junkp[:], featf[:], 1.0, pt["qnw"][:, j * H:(j + 1) * H],
                        OP.mult, OP.mult,
                        accum_out=qall[:, 2 * t + j: 2 * t + j + 1])
            qfin = sb.tile([128, PT * 2], F32, tag="qfin")
            qaddv = qadd_b[:].unsqueeze(1).broadcast_to((128, PT, 2))
            nc.vector.tensor_tensor(qfin[:].rearrange("p (t j) -> p t j", j=2),
                                    qall[:].rearrange("p (t j) -> p t j", j=2),
                                    qaddv, OP.add)
            nc.vector.tensor_tensor(qfin[:], qfin[:], m_t[:], OP.mult)
            nc.vector.tensor_tensor(qfin[:], qfin[:], cb_t[:], OP.add)
            nc.sync.dma_start(out_q[:], qfin[:])
      except _StopBuild:
        pass

    nc.compile()
    return nc


def _emit_window(X, msg_tiles, w, layer, chunk):
    """Aggregation + node update for one 128-node window."""
    nc, sb, T_w, pt = X.nc, X.sb, X.T_w, X.pt
    agg = X.psG.tile([128, H], F32, tag="agg")
    if layer == 0:
        nc.tensor.matmul(agg[:], X.xT[:, w * 128:(w + 1) * 128], pt["root0"][:],
                         start=True, stop=False)
        nc.tensor.matmul(agg[:], X.ones[:, :128], pt["bias0"][:],
                         start=False, stop=False)
    else:
        nc.tensor.matmul(agg[:], X.h1T_tiles[w][:], pt["root1"][:],
                         start=True, stop=False)
        nc.tensor.matmul(agg[:], X.ones[:, :128], pt["bias1"][:],
                         start=False, stop=False)
    for s in range(T_w):
        t = w * T_w + s
        oh = sb.tile([128, 128], F32, tag="oh")
        nc.sync.dma_start(oh[:], X.din["onehot"][t])
        nc.tensor.matmul(agg[:], oh[:], msg_tiles.pop(t)[:],
                         start=False, stop=(s == T_w - 1))
    hr = sb.tile([128, H], F32, tag="hrelu")
    nc.scalar.activation(hr[:], agg[:], ACTF.Relu)
    hw = X.hold.tile([128, H], F32, tag=f"hw{layer}")
    g_ap = pt["ln0g"][:] if layer == 0 else pt["ln1g"][:]
    b_ap = pt["ln0b"][:] if layer == 0 else pt["ln1b"][:]
    _emit_layernorm(nc, X.lnp, hr[:], g_ap, b_ap, hw[:], 128, H, X.eps)
    nc.sync.dma_start(chunk[w * 128:(w + 1) * 128, :], hw[:])
    if layer == 0:
        tr_ps = X.psT.tile([128, 128], F32, tag="psT")
        nc.tensor.transpose(tr_ps[:H, :], hw[:], pt["ident"][:])
        hT = X.hold.tile([H, 128], F32, tag="hT")
        nc.vector.tensor_copy(hT[:], tr_ps[:H, :])
        X.h1T_tiles.append(hT)
    else:
        nreal = min(128, NPC - w * 128)
        nc.gpsimd.tensor_reduce(X.gacc[:, w * H:(w + 1) * H], hw[:nreal, :],
                                axis=AX.C, op=OP.add)


def kernel(x, edge_index, edge_attr, pipe_edge_idx, pipe_open_mask, params):
    x = np.asarray(x, np.float32)
    edge_index = np.asarray(edge_index)
    edge_attr = np.asarray(edge_attr, np.float32)
    pipe_edge_idx = np.asarray(pipe_edge_idx)
    pipe_open_mask = np.asarray(pipe_open_mask, np.float32)

    cores, T_w, NT, E_pad = _host_prep(x, edge_index, edge_attr,
                                       pipe_edge_idx, pipe_open_mask)
    pp = _prep_params(params)

    nc = build_program(T_w, NT, E_pad, pp)

    in_maps = []
    for c in range(NCORES):
        m = dict(pp)
        m.update(cores[c])
        in_maps.append({k: np.ascontiguousarray(v) for k, v in m.items()})

    res = run_bass_kernel_spmd(nc, in_maps, core_ids=list(range(NCORES)))
    global LAST_RESULT
    LAST_RESULT = res

    qs = []
    for c in range(NCORES):
        oq = np.asarray(res.results[c]["out_q"]).reshape(128, PT, 2)
        qs.append(oq.transpose(1, 0, 2).reshape(PPAD, 2)[:PPC])
    q = np.concatenate(qs, axis=0)
    qg = np.asarray(res.results[0]["out_qg"]).reshape(1, 2)
    out = np.concatenate([q.reshape(1, -1), qg], axis=1).astype(np.float32)
    return out


# revision 15
# speedup vs baseline: 1.2878x; 1.2878x over previous
"""Trainium2 Bass kernel for the DQN-GNN (2-layer NNConv + DQN heads).

Strategy (8 NeuronCores, SPMD single program):
  - Host: sort edges by dst, shard edge-parallel by dst-ownership (1250
    nodes/core), pad each 128-node window to a uniform tile count, build
    inv-degree-scaled one-hot scatter matrices, int16 gather indices.
  - Device per core: edge-MLP via PE matmuls (aT computed transposed),
    per-edge matvec via DVE broadcast-multiply + strided reduce,
    scatter-mean via one-hot matmul accumulation in PSUM, node update
    (root/bias folded into same PSUM group), relu+LN, AllGather h,
    dma_gather for h[src]; same again for layer 2; DQN heads with PE
    transposes + fused scalar_tensor_tensor reductions.
  - Host: unshard pipe Q-values, concat with q_global.
"""

import numpy as np

import concourse.bass as bass
import concourse.mybir as mybir
import concourse.bacc as bacc
from concourse import tile
from concourse.bass_utils import run_bass_kernel_spmd

F32 = mybir.dt.float32
BF16 = mybir.dt.bfloat16
I16 = mybir.dt.int16
AX = mybir.AxisListType
OP = mybir.AluOpType
ACTF = mybir.ActivationFunctionType

# problem constants (hardcoded per harness contract)
N, E, P = 10000, 50000, 20000
NODE_IN, EDGE_IN, H = 4, 4, 64
EPS = 1e-5
NCORES = 8
NPC = N // NCORES          # 1250 nodes owned per core
NW = 10                    # 128-node windows per core
NPAD = NW * 128            # 1280 padded node slots per core
PPC = P // NCORES          # 2500 pipes per core
PT = (PPC + 127) // 128    # 20 pipe tiles
PPAD = PT * 128

LAST_RESULT = None
BUILD_STAGE = 99


def _pack_idx16(idx, total):
    """Pack indices into the (16, total//16) SWDGE layout: elem i at [i%16, i//16]."""
    flat = np.zeros(total, np.int64)
    flat[: len(idx)] = idx
    blk = flat.reshape(total // 16, 16).T.astype(np.int16)
    return np.ascontiguousarray(np.tile(blk, (8, 1)))


def _table_row(n):
    """Node id -> row in the all-gathered (NCORES*NPAD, H) h table."""
    return (n // NPC) * NPAD + (n % NPC)


def _host_prep(x, edge_index, edge_attr, pipe_edge_idx, pipe_open_mask):
    src = edge_index[0].astype(np.int64)
    dst = edge_index[1].astype(np.int64)
    deg = np.bincount(dst, minlength=N).astype(np.float32)
    inv_cnt = (1.0 / np.maximum(deg, 1.0)).astype(np.float32)

    order = np.argsort(dst, kind="stable")

    # per (core, window) edge lists (edges sorted by dst already)
    per_cw = [[None] * NW for _ in range(NCORES)]
    dst_s = dst[order]
    core_s = dst_s // NPC
    win_s = (dst_s - core_s * NPC) // 128
    for c in range(NCORES):
        mask_c = core_s == c
        for w in range(NW):
            per_cw[c][w] = order[mask_c & (win_s == w)]
    T_w = 1
    for c in range(NCORES):
        for w in range(NW):
            T_w = max(T_w, -(-len(per_cw[c][w]) // 128))
    NT = NW * T_w
    E_pad = NT * 128

    cores = []
    for c in range(NCORES):
        perm = np.full((NT, 128), -1, np.int64)
        for w in range(NW):
            lst = per_cw[c][w]
            for s, e in enumerate(lst):
                perm[w * T_w + s // 128, s % 128] = e
        flat = perm.reshape(-1)
        valid = flat >= 0
        fsafe = np.where(valid, flat, 0)

        onehot = np.zeros((NT, 128, 128), np.float32)
        t_idx, p_idx = np.nonzero(perm >= 0)
        e_ids = perm[t_idx, p_idx]
        loc = dst[e_ids] - c * NPC - (t_idx // T_w) * 128
        onehot[t_idx, p_idx, loc] = inv_cnt[dst[e_ids]]

        eaT = np.ascontiguousarray(
            np.where(valid[:, None], edge_attr[fsafe], 0.0).T.astype(np.float32))
        xsrc0 = np.where(valid[:, None], x[src[fsafe]], 0.0).astype(np.float32)
        xsrc0 = np.ascontiguousarray(
            xsrc0.reshape(NT, 128, NODE_IN).transpose(1, 0, 2))  # (128, NT, 4)
        gidx = _pack_idx16(np.where(valid, _table_row(src[fsafe]), 0), E_pad)

        xT = np.zeros((NODE_IN, NPAD), np.float32)
        xT[:, :NPC] = x[c * NPC:(c + 1) * NPC].T

        pids = np.arange(c * PPC, (c + 1) * PPC)
        pe = pipe_edge_idx[pids].astype(np.int64)
        gsu = _pack_idx16(_table_row(src[pe]), PPAD)
        gsv = _pack_idx16(_table_row(dst[pe]), PPAD)
        m = np.zeros((PPAD, 2), np.float32)
        m[:PPC, 0] = pipe_open_mask[pids]
        m[:PPC, 1] = 1.0 - pipe_open_mask[pids]
        cb = np.where(m > 0.5, np.float32(0.0), np.float32(-1e9))
        cb[PPC:] = 0.0
        m_t = np.ascontiguousarray(
            m.reshape(PT, 128, 2).transpose(1, 0, 2).reshape(128, PT * 2))
        cb_t = np.ascontiguousarray(
            cb.reshape(PT, 128, 2).transpose(1, 0, 2).reshape(128, PT * 2))

        cores.append(dict(
            onehot=onehot, eaT=eaT, xsrc0=xsrc0, gidx=gidx, xT=xT,
            gsu=gsu, gsv=gsv, m=m_t, cb=cb_t,
        ))
    return cores, T_w, NT, E_pad


def _bcast(v, parts=128):
    v = np.asarray(v, np.float32).reshape(1, -1)
    return np.ascontiguousarray(np.broadcast_to(v, (parts, v.shape[1])))


def _prep_params(params):
    c0, c1 = params["convs"][0], params["convs"][1]
    qn_w = np.asarray(params["qn_w"], np.float32)
    pp = dict(
        e1w0=np.asarray(c0["e1_w"], np.float32),
        e1b0=np.asarray(c0["e1_b"], np.float32).reshape(1, -1),
        e2w0=np.asarray(c0["e2_w"], np.float32),  # cast to bf16 at upload
        e2b0=np.asarray(c0["e2_b"], np.float32).reshape(1, -1),
        root0=np.asarray(c0["root"], np.float32),
        bias0=np.asarray(c0["bias"], np.float32).reshape(1, -1),
        ln0g=_bcast(c0["ln_g"]), ln0b=_bcast(c0["ln_b"]),
        e1w1=np.asarray(c1["e1_w"], np.float32),
        e1b1=np.asarray(c1["e1_b"], np.float32).reshape(1, -1),
        e2w1=np.asarray(c1["e2_w"], np.float32),
        e2b1=np.asarray(c1["e2_b"], np.float32).reshape(1, -1),
        root1=np.asarray(c1["root"], np.float32),
        bias1=np.asarray(c1["bias"], np.float32).reshape(1, -1),
        ln1g=_bcast(c1["ln_g"]), ln1b=_bcast(c1["ln_b"]),
        pairw=np.asarray(params["pair_w"], np.float32),
        pairb=_bcast(params["pair_b"]),
        plng=_bcast(params["pair_ln_g"]), plnb=_bcast(params["pair_ln_b"]),
        qnw=np.concatenate([_bcast(qn_w[:, j]) for j in range(2)], axis=1),
        qnb=np.asarray(params["qn_b"], np.float32).reshape(1, 2),
        ghw=np.asarray(params["gh_w"], np.float32),
        ghb=np.asarray(params["gh_b"], np.float32).reshape(1, -1),
        glng=np.asarray(params["gh_ln_g"], np.float32).reshape(1, -1),
        glnb=np.asarray(params["gh_ln_b"], np.float32).reshape(1, -1),
        qgw=np.ascontiguousarray(np.asarray(params["qg_w"], np.float32).T.reshape(1, 128)),
        qgb=np.asarray(params["qg_b"], np.float32).reshape(1, 2),
        ident=np.eye(128, dtype=np.float32),
    )
    return pp


PARAM_SHAPES = dict(
    e1w0=(4, 64), e1b0=(1, 64), e2w0=(64, 256), e2b0=(1, 256), root0=(4, 64),
    bias0=(1, 64), ln0g=(128, 64), ln0b=(128, 64),
    e1w1=(4, 128), e1b1=(1, 128), e2w1=(128, 4096), e2b1=(1, 4096), root1=(64, 64),
    bias1=(1, 64), ln1g=(128, 64), ln1b=(128, 64),
    pairw=(128, 64), pairb=(128, 64), plng=(128, 64), plnb=(128, 64),
    qnw=(128, 128), qnb=(1, 2),
    ghw=(64, 64), ghb=(1, 64), glng=(1, 64), glnb=(1, 64),
    qgw=(1, 128), qgb=(1, 2), ident=(128, 128),
)


def _emit_layernorm(nc, sb, x_ap, g_ap, b_ap, out_ap, parts, width, eps_ap):
    """out = (x - mean)/sqrt(var + EPS) * g + b, per-partition over free dim."""
    s = sb.tile([parts, 1], F32, tag="ln_s")
    mu = sb.tile([parts, 1], F32, tag="ln_mu")
    cen = sb.tile([parts, width], F32, tag="ln_cen")
    var = sb.tile([parts, 1], F32, tag="ln_var")
    junk = sb.tile([parts, width], F32, tag="ln_junk")
    std = sb.tile([parts, 1], F32, tag="ln_std")
    rstd = sb.tile([parts, 1], F32, tag="ln_rstd")
    nc.vector.reduce_sum(s[:], x_ap, axis=AX.X)
    nc.vector.tensor_scalar_mul(mu[:], s[:], 1.0 / width)
    nc.vector.tensor_scalar_sub(cen[:], x_ap, mu[:])
    nc.vector.tensor_tensor(junk[:], cen[:], cen[:], OP.mult)
    nc.vector.reduce_sum(var[:], junk[:], axis=AX.X)
    nc.scalar.activation(std[:], var[:], ACTF.Sqrt, bias=eps_ap[:parts, :],
                         scale=1.0 / width)
    nc.vector.reciprocal(rstd[:], std[:])
    nc.vector.scalar_tensor_tensor(out_ap, cen[:], rstd[:], g_ap, OP.mult, OP.mult)
    nc.vector.tensor_tensor(out_ap, out_ap, b_ap, OP.add)


class _Ctx:
    pass


class _StopBuild(Exception):
    pass


def build_program(T_w, NT, E_pad, pp):
    nc = bacc.Bacc("TRN2", target_bir_lowering=False, debug=False,
                   num_devices=NCORES)
    RG = [list(range(NCORES))]

    has_e1b0 = bool(np.any(pp["e1b0"] != 0))
    has_e1b1 = bool(np.any(pp["e1b1"] != 0))
    has_e2b0 = bool(np.any(pp["e2b0"] != 0))
    has_e2b1 = bool(np.any(pp["e2b1"] != 0))

    BF16_PARAMS = {"e2w0", "e2w1"}
    din = {}
    for name, shp in PARAM_SHAPES.items():
        dt = BF16 if name in BF16_PARAMS else F32
        din[name] = nc.dram_tensor(name, shp, dt, kind="ExternalInput")
    din["onehot"] = nc.dram_tensor("onehot", (NT, 128, 128), F32, kind="ExternalInput")
    din["eaT"] = nc.dram_tensor("eaT", (4, E_pad), F32, kind="ExternalInput")
    din["xsrc0"] = nc.dram_tensor("xsrc0", (128, NT, NODE_IN), F32, kind="ExternalInput")
    din["gidx"] = nc.dram_tensor("gidx", (128, E_pad // 16), I16, kind="ExternalInput")
    din["xT"] = nc.dram_tensor("xT", (NODE_IN, NPAD), F32, kind="ExternalInput")
    din["gsu"] = nc.dram_tensor("gsu", (128, PPAD // 16), I16, kind="ExternalInput")
    din["gsv"] = nc.dram_tensor("gsv", (128, PPAD // 16), I16, kind="ExternalInput")
    din["m"] = nc.dram_tensor("m", (128, PT * 2), F32, kind="ExternalInput")
    din["cb"] = nc.dram_tensor("cb", (128, PT * 2), F32, kind="ExternalInput")

    out_q = nc.dram_tensor("out_q", (128, PT * 2), F32, kind="ExternalOutput")
    out_qg = nc.dram_tensor("out_qg", (1, 2), F32, kind="ExternalOutput")

    h1chunk = nc.dram_tensor("h1chunk", (NPAD, H), F32, kind="Internal")
    h1full = nc.dram_tensor("h1full", (NCORES * NPAD, H), F32, kind="Internal",
                            addr_space="Shared")
    h2chunk = nc.dram_tensor("h2chunk", (NPAD, H), F32, kind="Internal")
    h2full = nc.dram_tensor("h2full", (NCORES * NPAD, H), F32, kind="Internal",
                            addr_space="Shared")
    gin = nc.dram_tensor("gin", (H, 1), F32, kind="Internal")
    gout = nc.dram_tensor("gout", (H, 1), F32, kind="Internal", addr_space="Shared")

    NG = -(-NT // 4)  # 512-edge groups

    with tile.TileContext(nc) as tc:
      try:
        with (
            tc.tile_pool(name="const", bufs=1) as cp,
            tc.tile_pool(name="sb", bufs=2) as sb,
            tc.tile_pool(name="lnp", bufs=2) as lnp,
            tc.tile_pool(name="hold", bufs=NW + 1) as hold,
            tc.tile_pool(name="msgs", bufs=T_w + 4) as msgp,
            tc.tile_pool(name="psA", bufs=1, space="PSUM") as psA,
            tc.tile_pool(name="psW", bufs=2, space="PSUM") as psW,
            tc.tile_pool(name="psG", bufs=1, space="PSUM") as psG,
            tc.tile_pool(name="psT", bufs=2, space="PSUM") as psT,
        ):
            X = _Ctx()
            X.nc, X.sb, X.lnp, X.hold = nc, sb, lnp, hold
            X.psG, X.psT, X.din = psG, psT, din
            X.T_w = T_w

            pt = {}
            for name, shp in PARAM_SHAPES.items():
                dt = BF16 if name in BF16_PARAMS else F32
                pt[name] = cp.tile(list(shp), dt, tag=f"c_{name}", name=f"ct_{name}")
                nc.sync.dma_start(pt[name][:], din[name][:])
            X.pt = pt
            ones_row = cp.tile([1, 512], F32, tag="ones_row")
            nc.vector.memset(ones_row[:], 1.0)
            X.ones = ones_row
            eps_t = cp.tile([128, 1], F32, tag="eps_t")
            nc.vector.memset(eps_t[:], EPS)
            X.eps = eps_t

            xsrc0 = cp.tile([128, NT, NODE_IN], F32, tag="xsrc0")
            nc.sync.dma_start(xsrc0[:], din["xsrc0"][:])
            gidx = cp.tile([128, E_pad // 16], I16, tag="gidx")
            nc.sync.dma_start(gidx[:], din["gidx"][:])
            xTt = cp.tile([NODE_IN, NPAD], F32, tag="xT")
            nc.sync.dma_start(xTt[:], din["xT"][:])
            X.xT = xTt

            X.h1T_tiles = []
            X.hw1_tiles = []

            # ================= layer 0 (conv0) =================
            msg_tiles = {}
            next_w = 0
            for g in range(NG):
                gt = min(4, NT - 4 * g)
                ncols = gt * 128
                ea_g = sb.tile([4, 512], F32, tag="ea_g")
                nc.sync.dma_start(ea_g[:, :ncols], din["eaT"][:, g * 512: g * 512 + ncols])
                aT0_ps = psA.tile([128, 512], F32, tag="aT_ps")
                nc.tensor.matmul(aT0_ps[:64, :ncols], pt["e1w0"][:], ea_g[:, :ncols],
                                 start=True, stop=not has_e1b0)
                if has_e1b0:
                    nc.tensor.matmul(aT0_ps[:64, :ncols], pt["e1b0"][:],
                                     ones_row[:, :ncols], start=False, stop=True)
                aT0 = sb.tile([64, 512], BF16, tag="aT0_sb")
                nc.scalar.activation(aT0[:, :ncols], aT0_ps[:64, :ncols], ACTF.Relu)

                for ti in range(gt):
                    t = 4 * g + ti
                    w0_ps = psW.tile([128, 1024], F32, tag="wq")
                    nc.tensor.matmul(w0_ps[:, :256], aT0[:, ti * 128:(ti + 1) * 128],
                                     pt["e2w0"][:], start=True, stop=not has_e2b0)
                    if has_e2b0:
                        nc.tensor.matmul(w0_ps[:, :256], ones_row[:, :128],
                                         pt["e2b0"][:], start=False, stop=True)
                    y0 = sb.tile([128, 1024], F32, tag="y")
                    y0v = y0[:, :256].rearrange("p (i o) -> p i o", o=H)
                    w0v = w0_ps[:, :256].rearrange("p (i o) -> p i o", o=H)
                    xv = xsrc0[:, t, :].unsqueeze(2).broadcast_to((128, NODE_IN, H))
                    nc.vector.tensor_tensor(y0v, w0v, xv, OP.mult)
                    msg = msgp.tile([128, H], F32, tag="msg")
                    nc.vector.reduce_sum(
                        msg[:], y0[:, :256].rearrange("p (i o) -> p o i", o=H),
                        axis=AX.X)
                    msg_tiles[t] = msg

                while next_w < NW and 4 * g + gt >= (next_w + 1) * T_w:
                    _emit_window(X, msg_tiles, next_w, layer=0, chunk=h1chunk)
                    next_w += 1

            # ---- AllGather h1, gather h1[src] ----
            if BUILD_STAGE < 2:
                raise _StopBuild()
            nc.gpsimd.collective_compute(
                "AllGather", OP.bypass, replica_groups=RG,
                ins=[h1chunk[:]], outs=[h1full[:]])
            hs1 = cp.tile([128, NT, H], F32, tag="hs1")
            for b in range(0, E_pad, 1024):
                nn = min(1024, E_pad - b)
                nc.gpsimd.dma_gather(
                    hs1[:, b // 128:(b + nn) // 128, :], h1full[:],
                    gidx[:, b // 16:(b + nn) // 16], nn, nn, H)

            # ================= layer 1 (conv1) =================
            if BUILD_STAGE < 3:
                raise _StopBuild()
            msg_tiles1 = {}
            next_w = 0
            for g in range(NG):
                gt = min(4, NT - 4 * g)
                ncols = gt * 128
                ea_g = sb.tile([4, 512], F32, tag="ea_g")
                nc.sync.dma_start(ea_g[:, :ncols], din["eaT"][:, g * 512: g * 512 + ncols])
                aT1_ps = psA.tile([128, 512], F32, tag="aT_ps")
                nc.tensor.matmul(aT1_ps[:, :ncols], pt["e1w1"][:], ea_g[:, :ncols],
                                 start=True, stop=not has_e1b1)
                if has_e1b1:
                    nc.tensor.matmul(aT1_ps[:, :ncols], pt["e1b1"][:],
                                     ones_row[:, :ncols], start=False, stop=True)
                aT1 = sb.tile([128, 512], BF16, tag="aT1_sb")
                nc.scalar.activation(aT1[:, :ncols], aT1_ps[:, :ncols], ACTF.Relu)

                for ti in range(gt):
                    t = 4 * g + ti
                    msgq = sb.tile([128, H, 4], F32, tag="msgq")
                    for q in range(4):
                        w1q = psW.tile([128, 1024], F32, tag="wq")
                        for hf in range(2):
                            lo = q * 1024 + hf * 512
                            nc.tensor.matmul(
                                w1q[:, hf * 512:(hf + 1) * 512],
                                aT1[:, ti * 128:(ti + 1) * 128],
                                pt["e2w1"][:, lo: lo + 512],
                                start=True, stop=not has_e2b1)
                            if has_e2b1:
                                nc.tensor.matmul(
                                    w1q[:, hf * 512:(hf + 1) * 512],
                                    ones_row[:, :128],
                                    pt["e2b1"][:, lo: lo + 512],
                                    start=False, stop=True)
                        y = sb.tile([128, 1024], BF16, tag="y")
                        yv = y[:].rearrange("p (i o) -> p i o", o=H)
                        w1v = w1q[:].rearrange("p (i o) -> p i o", o=H)
                        hv = hs1[:, t, q * 16:(q + 1) * 16].unsqueeze(2).broadcast_to(
                            (128, 16, H))
                        nc.vector.tensor_tensor(yv, w1v, hv, OP.mult)
                        # bf16 pairwise tree over i (16 -> 8 -> 4 -> 2 -> 1)
                        t1 = sb.tile([128, 512], BF16, tag="tr1")
                        nc.vector.tensor_tensor(t1[:], y[:, :512], y[:, 512:], OP.add)
                        t2 = sb.tile([128, 256], BF16, tag="tr2")
                        nc.vector.tensor_tensor(t2[:], t1[:, :256], t1[:, 256:], OP.add)
                        t3 = sb.tile([128, 128], BF16, tag="tr3")
                        nc.vector.tensor_tensor(t3[:], t2[:, :128], t2[:, 128:], OP.add)
                        nc.vector.tensor_tensor(
                            msgq[:, :, q], t3[:, :64], t3[:, 64:], OP.add)
                    msg = msgp.tile([128, H], F32, tag="msg")
                    nc.vector.reduce_sum(msg[:], msgq[:], axis=AX.X)
                    msg_tiles1[t] = msg

                while next_w < NW and 4 * g + gt >= (next_w + 1) * T_w:
                    _emit_window(X, msg_tiles1, next_w, layer=1, chunk=h2chunk)
                    next_w += 1

            # ---- AllGather h2 + AllReduce g ----
            if BUILD_STAGE < 4:
                raise _StopBuild()
            nc.gpsimd.collective_compute(
                "AllGather", OP.bypass, replica_groups=RG,
                ins=[h2chunk[:]], outs=[h2full[:]])
            ones_col = cp.tile([128, 1], F32, tag="ones_col")
            nc.vector.memset(ones_col[:], 1.0)
            g_ps = psT.tile([128, 128], F32, tag="psT")
            for w in range(NW):
                nreal = min(128, NPC - w * 128)
                nc.tensor.matmul(g_ps[:H, :1], X.hw1_tiles[w][:nreal, :],
                                 ones_col[:nreal, :],
                                 start=(w == 0), stop=(w == NW - 1))
            gsum = sb.tile([H, 1], F32, tag="gsum")
            nc.vector.tensor_copy(gsum[:], g_ps[:H, :1])
            nc.sync.dma_start(gin[:], gsum[:])
            nc.gpsimd.collective_compute(
                "AllReduce", OP.add, replica_groups=RG,
                ins=[gin[:]], outs=[gout[:]])

            # ---- global head (redundant on every core) ----
            if BUILD_STAGE < 5:
                raise _StopBuild()
            gcol = sb.tile([H, 1], F32, tag="gcol")
            nc.sync.dma_start(gcol[:], gout[:])
            gcols = sb.tile([H, 1], F32, tag="gcols")
            nc.vector.tensor_scalar_mul(gcols[:], gcol[:], 1.0 / N)
            ghp = psT.tile([128, 128], F32, tag="psT")
            nc.tensor.matmul(ghp[:1, :H], gcols[:], pt["ghw"][:], start=True, stop=False)
            nc.tensor.matmul(ghp[:1, :H], ones_row[:, :1], pt["ghb"][:],
                             start=False, stop=True)
            ghr = sb.tile([1, H], F32, tag="ghr")
            nc.scalar.activation(ghr[:], ghp[:1, :H], ACTF.Relu)
            ghf = sb.tile([1, H], F32, tag="ghf")
            _emit_layernorm(nc, lnp, ghr[:], pt["glng"][:], pt["glnb"][:], ghf[:], 1, H, eps_t)
            qgl = sb.tile([1, 2], F32, tag="qgl")
            junkg = sb.tile([1, H], F32, tag="junkg")
            for j in range(2):
                nc.vector.scalar_tensor_tensor(
                    junkg[:], ghf[:], 1.0, pt["qgw"][:, j * H:(j + 1) * H], OP.mult, OP.mult,
                    accum_out=qgl[:, j:j + 1])
            qglo = sb.tile([1, 2], F32, tag="qglo")
            nc.vector.tensor_tensor(qglo[:], qgl[:], pt["qgb"][:], OP.add)
            nc.sync.dma_start(out_qg[:], qglo[:])
            qadd = sb.tile([1, 2], F32, tag="qadd")
            nc.vector.tensor_tensor(qadd[:], qglo[:], pt["qnb"][:], OP.add)
            qadd_ps = psT.tile([128, 128], F32, tag="psT")
            nc.tensor.matmul(qadd_ps[:, :2], ones_row[:, :128], qadd[:],
                             start=True, stop=True)
            qadd_b = sb.tile([128, 2], F32, tag="qadd_b")
            nc.vector.tensor_copy(qadd_b[:], qadd_ps[:, :2])

            # ---- pipe head ----
            if BUILD_STAGE < 6:
                raise _StopBuild()
            pu = cp.tile([128, PT, H], F32, tag="pu")
            pv = cp.tile([128, PT, H], F32, tag="pv")
            gsu_t = cp.tile([128, PPAD // 16], I16, tag="gsu")
            gsv_t = cp.tile([128, PPAD // 16], I16, tag="gsv")
            nc.sync.dma_start(gsu_t[:], din["gsu"][:])
            nc.sync.dma_start(gsv_t[:], din["gsv"][:])
            for b in range(0, PPAD, 1024):
                nn = min(1024, PPAD - b)
                nc.gpsimd.dma_gather(
                    pu[:, b // 128:(b + nn) // 128, :], h2full[:],
                    gsu_t[:, b // 16:(b + nn) // 16], nn, nn, H)
                nc.gpsimd.dma_gather(
                    pv[:, b // 128:(b + nn) // 128, :], h2full[:],
                    gsv_t[:, b // 16:(b + nn) // 16], nn, nn, H)

            m_t = cp.tile([128, PT * 2], F32, tag="m_t")
            cb_t = cp.tile([128, PT * 2], F32, tag="cb_t")
            nc.sync.dma_start(m_t[:], din["m"][:])
            nc.sync.dma_start(cb_t[:], din["cb"][:])

            qall = cp.tile([128, PT * 2], F32, tag="qall")
            for t in range(PT):
                pairT = sb.tile([128, 128], F32, tag="pairT")
                for half, srct in ((0, pu), (1, pv)):
                    tr_ps = psT.tile([128, 128], F32, tag="psT")
                    nc.tensor.transpose(tr_ps[:H, :], srct[:, t, :], pt["ident"][:])
                    nc.vector.tensor_copy(pairT[half * H:(half + 1) * H, :],
                                          tr_ps[:H, :])
                feat_ps = psT.tile([128, 128], F32, tag="psT")
                nc.tensor.matmul(feat_ps[:, :H], pairT[:], pt["pairw"][:],
                                 start=True, stop=True)
                featb = sb.tile([128, H], F32, tag="featb")
                nc.vector.tensor_tensor(featb[:], feat_ps[:, :H], pt["pairb"][:],
                                        OP.add)
                featr = sb.tile([128, H], F32, tag="featr")
                nc.scalar.activation(featr[:], featb[:], ACTF.Relu)
                featf = sb.tile([128, H], F32, tag="featf")
                _emit_layernorm(nc, lnp, featr[:], pt["plng"][:], pt["plnb"][:],
                                featf[:], 128, H, eps_t)
                junkp = sb.tile([128, H], F32, tag="junkp")
                for j in range(2):
                    nc.vector.scalar_tensor_tensor(
                        junkp[:], featf[:], 1.0, pt["qnw"][:, j * H:(j + 1) * H],
                        OP.mult, OP.mult,
                        accum_out=qall[:, 2 * t + j: 2 * t + j + 1])
            qfin = sb.tile([128, PT * 2], F32, tag="qfin")
            qaddv = qadd_b[:].unsqueeze(1).broadcast_to((128, PT, 2))
            nc.vector.tensor_tensor(qfin[:].rearrange("p (t j) -> p t j", j=2),
                                    qall[:].rearrange("p (t j) -> p t j", j=2),
                                    qaddv, OP.add)
            nc.vector.tensor_tensor(qfin[:], qfin[:], m_t[:], OP.mult)
            nc.vector.tensor_tensor(qfin[:], qfin[:], cb_t[:], OP.add)
            nc.sync.dma_start(out_q[:], qfin[:])
      except _StopBuild:
        pass

    nc.compile()
    return nc


def _emit_window(X, msg_tiles, w, layer, chunk):
    """Aggregation + node update for one 128-node window."""
    nc, sb, T_w, pt = X.nc, X.sb, X.T_w, X.pt
    agg = X.psG.tile([128, H], F32, tag="agg")
    if layer == 0:
        nc.tensor.matmul(agg[:], X.xT[:, w * 128:(w + 1) * 128], pt["root0"][:],
                         start=True, stop=False)
        nc.tensor.matmul(agg[:], X.ones[:, :128], pt["bias0"][:],
                         start=False, stop=False)
    else:
        nc.tensor.matmul(agg[:], X.h1T_tiles[w][:], pt["root1"][:],
                         start=True, stop=False)
        nc.tensor.matmul(agg[:], X.ones[:, :128], pt["bias1"][:],
                         start=False, stop=False)
    for s in range(T_w):
        t = w * T_w + s
        oh = sb.tile([128, 128], F32, tag="oh")
        nc.sync.dma_start(oh[:], X.din["onehot"][t])
        nc.tensor.matmul(agg[:], oh[:], msg_tiles.pop(t)[:],
                         start=False, stop=(s == T_w - 1))
    hr = sb.tile([128, H], F32, tag="hrelu")
    nc.scalar.activation(hr[:], agg[:], ACTF.Relu)
    hw = X.hold.tile([128, H], F32, tag=f"hw{layer}")
    g_ap = pt["ln0g"][:] if layer == 0 else pt["ln1g"][:]
    b_ap = pt["ln0b"][:] if layer == 0 else pt["ln1b"][:]
    _emit_layernorm(nc, X.lnp, hr[:], g_ap, b_ap, hw[:], 128, H, X.eps)
    nc.sync.dma_start(chunk[w * 128:(w + 1) * 128, :], hw[:])
    if layer == 0:
        tr_ps = X.psT.tile([128, 128], F32, tag="psT")
        nc.tensor.transpose(tr_ps[:H, :], hw[:], pt["ident"][:])
        hT = X.hold.tile([H, 128], F32, tag="hT")
        nc.vector.tensor_copy(hT[:], tr_ps[:H, :])
        X.h1T_tiles.append(hT)
    else:
        X.hw1_tiles.append(hw)


def kernel(x, edge_index, edge_attr, pipe_edge_idx, pipe_open_mask, params):
    x = np.asarray(x, np.float32)
    edge_index = np.asarray(edge_index)
    edge_attr = np.asarray(edge_attr, np.float32)
    pipe_edge_idx = np.asarray(pipe_edge_idx)
    pipe_open_mask = np.asarray(pipe_open_mask, np.float32)

    cores, T_w, NT, E_pad = _host_prep(x, edge_index, edge_attr,
                                       pipe_edge_idx, pipe_open_mask)
    pp = _prep_params(params)

    nc = build_program(T_w, NT, E_pad, pp)

    import ml_dtypes
    in_maps = []
    for c in range(NCORES):
        m = dict(pp)
        m.update(cores[c])
        mm = {}
        for k, v in m.items():
            if k in ("e2w0", "e2w1"):
                v = np.asarray(v).astype(ml_dtypes.bfloat16)
            mm[k] = np.ascontiguousarray(v)
        in_maps.append(mm)

    res = run_bass_kernel_spmd(nc, in_maps, core_ids=list(range(NCORES)))
    global LAST_RESULT
    LAST_RESULT = res

    qs = []
    for c in range(NCORES):
        oq = np.asarray(res.results[c]["out_q"]).reshape(128, PT, 2)
        qs.append(oq.transpose(1, 0, 2).reshape(PPAD, 2)[:PPC])
    q = np.concatenate(qs, axis=0)
    qg = np.asarray(res.results[0]["out_qg"]).reshape(1, 2)
    out = np.concatenate([q.reshape(1, -1), qg], axis=1).astype(np.float32)
    return out


# revision 21
# speedup vs baseline: 1.4724x; 1.1434x over previous
"""Trainium2 Bass kernel for the DQN-GNN (2-layer NNConv + DQN heads).

Strategy (8 NeuronCores, SPMD single program):
  - Host: sort edges by dst, shard edge-parallel by dst-ownership (1250
    nodes/core), pad each 128-node window to a uniform tile count, build
    inv-degree-scaled one-hot scatter matrices, int16 gather indices.
  - Device per core: edge-MLP via PE matmuls (aT computed transposed),
    per-edge matvec via DVE broadcast-multiply + strided reduce,
    scatter-mean via one-hot matmul accumulation in PSUM, node update
    (root/bias folded into same PSUM group), relu+LN, AllGather h,
    dma_gather for h[src]; same again for layer 2; DQN heads with PE
    transposes + fused scalar_tensor_tensor reductions.
  - Host: unshard pipe Q-values, concat with q_global.
"""

import numpy as np

import concourse.bass as bass
import concourse.mybir as mybir
import concourse.bacc as bacc
from concourse import tile
from concourse.bass_utils import run_bass_kernel_spmd

F32 = mybir.dt.float32
BF16 = mybir.dt.bfloat16
I16 = mybir.dt.int16
AX = mybir.AxisListType
OP = mybir.AluOpType
ACTF = mybir.ActivationFunctionType

# problem constants (hardcoded per harness contract)
N, E, P = 10000, 50000, 20000
NODE_IN, EDGE_IN, H = 4, 4, 64
EPS = 1e-5
NCORES = 8
NPC = N // NCORES          # 1250 nodes owned per core
NW = 10                    # 128-node windows per core
NPAD = NW * 128            # 1280 padded node slots per core
PPC = P // NCORES          # 2500 pipes per core
PT = (PPC + 127) // 128    # 20 pipe tiles
PPAD = PT * 128

LAST_RESULT = None
BUILD_STAGE = 99


def _pack_idx16(idx, total):
    """Pack indices into the (16, total//16) SWDGE layout: elem i at [i%16, i//16]."""
    flat = np.zeros(total, np.int64)
    flat[: len(idx)] = idx
    blk = flat.reshape(total // 16, 16).T.astype(np.int16)
    return np.ascontiguousarray(np.tile(blk, (8, 1)))


def _table_row(n):
    """Node id -> row in the all-gathered (NCORES*NPAD, H) h table."""
    return (n // NPC) * NPAD + (n % NPC)


def _host_prep(x, edge_index, edge_attr, pipe_edge_idx, pipe_open_mask):
    src = edge_index[0].astype(np.int64)
    dst = edge_index[1].astype(np.int64)
    deg = np.bincount(dst, minlength=N).astype(np.float32)
    inv_cnt = (1.0 / np.maximum(deg, 1.0)).astype(np.float32)

    order = np.argsort(dst, kind="stable")

    # per (core, window) edge lists (edges sorted by dst already)
    per_cw = [[None] * NW for _ in range(NCORES)]
    dst_s = dst[order]
    core_s = dst_s // NPC
    win_s = (dst_s - core_s * NPC) // 128
    for c in range(NCORES):
        mask_c = core_s == c
        for w in range(NW):
            per_cw[c][w] = order[mask_c & (win_s == w)]
    T_w = 1
    for c in range(NCORES):
        for w in range(NW):
            T_w = max(T_w, -(-len(per_cw[c][w]) // 128))
    NT = NW * T_w
    E_pad = NT * 128

    cores = []
    for c in range(NCORES):
        perm = np.full((NT, 128), -1, np.int64)
        for w in range(NW):
            lst = per_cw[c][w]
            for s, e in enumerate(lst):
                perm[w * T_w + s // 128, s % 128] = e
        flat = perm.reshape(-1)
        valid = flat >= 0
        fsafe = np.where(valid, flat, 0)

        onehot = np.zeros((NT, 128, 128), np.float32)
        t_idx, p_idx = np.nonzero(perm >= 0)
        e_ids = perm[t_idx, p_idx]
        loc = dst[e_ids] - c * NPC - (t_idx // T_w) * 128
        onehot[t_idx, p_idx, loc] = inv_cnt[dst[e_ids]]

        eaT = np.ascontiguousarray(
            np.where(valid[:, None], edge_attr[fsafe], 0.0).T.astype(np.float32))
        xsrc0 = np.where(valid[:, None], x[src[fsafe]], 0.0).astype(np.float32)
        xsrc0 = np.ascontiguousarray(
            xsrc0.reshape(NT, 128, NODE_IN).transpose(1, 0, 2))  # (128, NT, 4)
        gidx = _pack_idx16(np.where(valid, _table_row(src[fsafe]), 0), E_pad)

        xT = np.zeros((NODE_IN, NPAD), np.float32)
        xT[:, :NPC] = x[c * NPC:(c + 1) * NPC].T

        pids = np.arange(c * PPC, (c + 1) * PPC)
        pe = pipe_edge_idx[pids].astype(np.int64)
        gsu = _pack_idx16(_table_row(src[pe]), PPAD)
        gsv = _pack_idx16(_table_row(dst[pe]), PPAD)
        m = np.zeros((PPAD, 2), np.float32)
        m[:PPC, 0] = pipe_open_mask[pids]
        m[:PPC, 1] = 1.0 - pipe_open_mask[pids]
        cb = np.where(m > 0.5, np.float32(0.0), np.float32(-1e9))
        cb[PPC:] = 0.0
        m_t = np.ascontiguousarray(
            m.reshape(PT, 128, 2).transpose(1, 0, 2).reshape(128, PT * 2))
        cb_t = np.ascontiguousarray(
            cb.reshape(PT, 128, 2).transpose(1, 0, 2).reshape(128, PT * 2))

        cores.append(dict(
            onehot=onehot, eaT=eaT, xsrc0=xsrc0, gidx=gidx, xT=xT,
            gsu=gsu, gsv=gsv, m=m_t, cb=cb_t,
        ))
    return cores, T_w, NT, E_pad


def _bcast(v, parts=128):
    v = np.asarray(v, np.float32).reshape(1, -1)
    return np.ascontiguousarray(np.broadcast_to(v, (parts, v.shape[1])))


def _prep_params(params):
    c0, c1 = params["convs"][0], params["convs"][1]
    qn_w = np.asarray(params["qn_w"], np.float32)
    pp = dict(
        e1w0=np.asarray(c0["e1_w"], np.float32),
        e1b0=np.asarray(c0["e1_b"], np.float32).reshape(1, -1),
        e2w0=np.asarray(c0["e2_w"], np.float32),  # cast to bf16 at upload
        e2b0=np.asarray(c0["e2_b"], np.float32).reshape(1, -1),
        root0=np.asarray(c0["root"], np.float32),
        bias0=np.asarray(c0["bias"], np.float32).reshape(1, -1),
        ln0g=_bcast(c0["ln_g"]), ln0b=_bcast(c0["ln_b"]),
        e1w1=np.asarray(c1["e1_w"], np.float32),
        e1b1=np.asarray(c1["e1_b"], np.float32).reshape(1, -1),
        e2w1=np.asarray(c1["e2_w"], np.float32),
        e2b1=np.asarray(c1["e2_b"], np.float32).reshape(1, -1),
        root1=np.asarray(c1["root"], np.float32),
        bias1=np.asarray(c1["bias"], np.float32).reshape(1, -1),
        ln1g=_bcast(c1["ln_g"]), ln1b=_bcast(c1["ln_b"]),
        pairw=np.asarray(params["pair_w"], np.float32),
        pairb=_bcast(params["pair_b"]),
        plng=_bcast(params["pair_ln_g"]), plnb=_bcast(params["pair_ln_b"]),
        qnw=np.concatenate([_bcast(qn_w[:, j]) for j in range(2)], axis=1),
        qnb=np.asarray(params["qn_b"], np.float32).reshape(1, 2),
        ghw=np.asarray(params["gh_w"], np.float32),
        ghb=np.asarray(params["gh_b"], np.float32).reshape(1, -1),
        glng=np.asarray(params["gh_ln_g"], np.float32).reshape(1, -1),
        glnb=np.asarray(params["gh_ln_b"], np.float32).reshape(1, -1),
        qgw=np.ascontiguousarray(np.asarray(params["qg_w"], np.float32).T.reshape(1, 128)),
        qgb=np.asarray(params["qg_b"], np.float32).reshape(1, 2),
        ident=np.eye(128, dtype=np.float32),
    )
    return pp


PARAM_SHAPES = dict(
    e1w0=(4, 64), e1b0=(1, 64), e2w0=(64, 256), e2b0=(1, 256), root0=(4, 64),
    bias0=(1, 64), ln0g=(128, 64), ln0b=(128, 64),
    e1w1=(4, 128), e1b1=(1, 128), e2w1=(128, 4096), e2b1=(1, 4096), root1=(64, 64),
    bias1=(1, 64), ln1g=(128, 64), ln1b=(128, 64),
    pairw=(128, 64), pairb=(128, 64), plng=(128, 64), plnb=(128, 64),
    qnw=(128, 128), qnb=(1, 2),
    ghw=(64, 64), ghb=(1, 64), glng=(1, 64), glnb=(1, 64),
    qgw=(1, 128), qgb=(1, 2), ident=(128, 128),
)


def _emit_layernorm(nc, sb, x_ap, g_ap, b_ap, out_ap, parts, width, eps_ap):
    """out = (x - mean)/sqrt(var + EPS) * g + b, per-partition over free dim."""
    s = sb.tile([parts, 1], F32, tag="ln_s")
    mu = sb.tile([parts, 1], F32, tag="ln_mu")
    cen = sb.tile([parts, width], F32, tag="ln_cen")
    var = sb.tile([parts, 1], F32, tag="ln_var")
    junk = sb.tile([parts, width], F32, tag="ln_junk")
    std = sb.tile([parts, 1], F32, tag="ln_std")
    rstd = sb.tile([parts, 1], F32, tag="ln_rstd")
    nc.vector.reduce_sum(s[:], x_ap, axis=AX.X)
    nc.vector.tensor_scalar_mul(mu[:], s[:], 1.0 / width)
    nc.vector.tensor_scalar_sub(cen[:], x_ap, mu[:])
    nc.vector.tensor_tensor(junk[:], cen[:], cen[:], OP.mult)
    nc.vector.reduce_sum(var[:], junk[:], axis=AX.X)
    nc.scalar.activation(std[:], var[:], ACTF.Sqrt, bias=eps_ap[:parts, :],
                         scale=1.0 / width)
    nc.vector.reciprocal(rstd[:], std[:])
    nc.vector.scalar_tensor_tensor(out_ap, cen[:], rstd[:], g_ap, OP.mult, OP.mult)
    nc.vector.tensor_tensor(out_ap, out_ap, b_ap, OP.add)


def _emit_layernorm_b(nc, sb, x3, g_b, b_b, out3, parts, G, F, eps_ap, tagp=""):
    """Batched LN over G groups of F: x3/out3 are (parts, G, F) APs."""
    s = sb.tile([parts, G], F32, tag=f"lb_s{tagp}")
    mu = sb.tile([parts, G], F32, tag=f"lb_mu{tagp}")
    cen = sb.tile([parts, G, F], F32, tag=f"lb_cen{tagp}")
    junk = sb.tile([parts, G, F], F32, tag=f"lb_junk{tagp}")
    var = sb.tile([parts, G], F32, tag=f"lb_var{tagp}")
    std = sb.tile([parts, G], F32, tag=f"lb_std{tagp}")
    rstd = sb.tile([parts, G], F32, tag=f"lb_rstd{tagp}")
    nc.vector.reduce_sum(s[:], x3, axis=AX.X)
    nc.vector.tensor_scalar_mul(mu[:], s[:], 1.0 / F)
    nc.vector.tensor_tensor(cen[:], x3,
                            mu[:].unsqueeze(2).broadcast_to((parts, G, F)),
                            OP.subtract)
    nc.vector.tensor_tensor(junk[:], cen[:], cen[:], OP.mult)
    nc.vector.reduce_sum(var[:], junk[:], axis=AX.X)
    nc.scalar.activation(std[:], var[:], ACTF.Sqrt, bias=eps_ap[:parts, :],
                         scale=1.0 / F)
    nc.vector.reciprocal(rstd[:], std[:])
    nc.vector.tensor_tensor(cen[:], cen[:],
                            rstd[:].unsqueeze(2).broadcast_to((parts, G, F)),
                            OP.mult)
    nc.vector.tensor_tensor(cen[:], cen[:],
                            g_b[:].unsqueeze(1).broadcast_to((parts, G, F)),
                            OP.mult)
    nc.vector.tensor_tensor(out3, cen[:],
                            b_b[:].unsqueeze(1).broadcast_to((parts, G, F)),
                            OP.add)


class _Ctx:
    pass


class _StopBuild(Exception):
    pass


def build_program(T_w, NT, E_pad, pp):
    nc = bacc.Bacc("TRN2", target_bir_lowering=False, debug=False,
                   num_devices=NCORES)
    RG = [list(range(NCORES))]

    has_bias0 = bool(np.any(pp["bias0"] != 0))
    has_bias1 = bool(np.any(pp["bias1"] != 0))
    has_e1b0 = bool(np.any(pp["e1b0"] != 0))
    has_e1b1 = bool(np.any(pp["e1b1"] != 0))
    has_e2b0 = bool(np.any(pp["e2b0"] != 0))
    has_e2b1 = bool(np.any(pp["e2b1"] != 0))

    BF16_PARAMS = {"e2w0", "e2w1", "e1w0", "e1w1", "e1b0", "e1b1"}
    din = {}
    for name, shp in PARAM_SHAPES.items():
        dt = BF16 if name in BF16_PARAMS else F32
        din[name] = nc.dram_tensor(name, shp, dt, kind="ExternalInput")
    din["onehot"] = nc.dram_tensor("onehot", (NT, 128, 128), F32, kind="ExternalInput")
    din["eaT"] = nc.dram_tensor("eaT", (4, E_pad), BF16, kind="ExternalInput")
    din["xsrc0"] = nc.dram_tensor("xsrc0", (128, NT, NODE_IN), F32, kind="ExternalInput")
    din["gidx"] = nc.dram_tensor("gidx", (128, E_pad // 16), I16, kind="ExternalInput")
    din["xT"] = nc.dram_tensor("xT", (NODE_IN, NPAD), F32, kind="ExternalInput")
    din["gsu"] = nc.dram_tensor("gsu", (128, PPAD // 16), I16, kind="ExternalInput")
    din["gsv"] = nc.dram_tensor("gsv", (128, PPAD // 16), I16, kind="ExternalInput")
    din["m"] = nc.dram_tensor("m", (128, PT * 2), F32, kind="ExternalInput")
    din["cb"] = nc.dram_tensor("cb", (128, PT * 2), F32, kind="ExternalInput")

    out_q = nc.dram_tensor("out_q", (128, PT * 2), F32, kind="ExternalOutput")
    out_qg = nc.dram_tensor("out_qg", (1, 2), F32, kind="ExternalOutput")

    h1chunk = nc.dram_tensor("h1chunk", (NPAD, H), F32, kind="Internal")
    h1full = nc.dram_tensor("h1full", (NCORES * NPAD, H), F32, kind="Internal",
                            addr_space="Shared")
    h2chunk = nc.dram_tensor("h2chunk", (NPAD, H), F32, kind="Internal")
    h2full = nc.dram_tensor("h2full", (NCORES * NPAD, H), F32, kind="Internal",
                            addr_space="Shared")
    gin = nc.dram_tensor("gin", (H, 1), F32, kind="Internal")
    gout = nc.dram_tensor("gout", (H, 1), F32, kind="Internal", addr_space="Shared")

    NG = -(-NT // 4)  # 512-edge groups

    with tile.TileContext(nc) as tc:
      try:
        with (
            tc.tile_pool(name="const", bufs=1) as cp,
            tc.tile_pool(name="sb", bufs=2) as sb,
            tc.tile_pool(name="lnp", bufs=2) as lnp,
            tc.tile_pool(name="hold", bufs=NW + 1) as hold,
            tc.tile_pool(name="msgs", bufs=T_w + 4) as msgp,
            tc.tile_pool(name="psA", bufs=1, space="PSUM") as psA,
            tc.tile_pool(name="psW", bufs=2, space="PSUM") as psW,
            tc.tile_pool(name="psG", bufs=1, space="PSUM") as psG,
            tc.tile_pool(name="psT", bufs=2, space="PSUM") as psT,
        ):
            X = _Ctx()
            X.nc, X.sb, X.lnp, X.hold = nc, sb, lnp, hold
            X.cp = cp
            X.has_bias0, X.has_bias1 = has_bias0, has_bias1
            X.psG, X.psT, X.din = psG, psT, din
            X.T_w = T_w

            pt = {}
            for name, shp in PARAM_SHAPES.items():
                dt = BF16 if name in BF16_PARAMS else F32
                pt[name] = cp.tile(list(shp), dt, tag=f"c_{name}", name=f"ct_{name}")
                nc.sync.dma_start(pt[name][:], din[name][:])
            X.pt = pt
            ones_row = cp.tile([1, 512], F32, tag="ones_row")
            nc.vector.memset(ones_row[:], 1.0)
            X.ones = ones_row
            eps_t = cp.tile([128, 1], F32, tag="eps_t")
            nc.vector.memset(eps_t[:], EPS)
            X.eps = eps_t

            xsrc0 = cp.tile([128, NT, NODE_IN], F32, tag="xsrc0")
            nc.sync.dma_start(xsrc0[:], din["xsrc0"][:])
            gidx = cp.tile([128, E_pad // 16], I16, tag="gidx")
            nc.sync.dma_start(gidx[:], din["gidx"][:])
            xTt = cp.tile([NODE_IN, NPAD], F32, tag="xT")
            nc.sync.dma_start(xTt[:], din["xT"][:])
            X.xT = xTt

            X.h1T_tiles = []
            hr0_all = cp.tile([128, NW, H], F32, tag="hr0_all")
            hr1_all = cp.tile([128, NW, H], F32, tag="hr1_all")

            # ================= layer 0 (conv0) =================
            msg_tiles = {}
            next_w = 0
            for g in range(NG):
                gt = min(4, NT - 4 * g)
                ncols = gt * 128
                ea_g = sb.tile([4, 512], BF16, tag="ea_g")
                nc.sync.dma_start(ea_g[:, :ncols], din["eaT"][:, g * 512: g * 512 + ncols])
                aT0_ps = psA.tile([128, 512], F32, tag="aT_ps")
                nc.tensor.matmul(aT0_ps[:64, :ncols], pt["e1w0"][:], ea_g[:, :ncols],
                                 start=True, stop=not has_e1b0)
                if has_e1b0:
                    nc.tensor.matmul(aT0_ps[:64, :ncols], pt["e1b0"][:],
                                     ones_row[:, :ncols], start=False, stop=True)
                aT0 = sb.tile([64, 512], BF16, tag="aT0_sb")
                nc.scalar.activation(aT0[:, :ncols], aT0_ps[:64, :ncols], ACTF.Relu)

                for ti in range(gt):
                    t = 4 * g + ti
                    w0_ps = psW.tile([128, 1024], F32, tag="wq")
                    nc.tensor.matmul(w0_ps[:, :256], aT0[:, ti * 128:(ti + 1) * 128],
                                     pt["e2w0"][:], start=True, stop=not has_e2b0)
                    if has_e2b0:
                        nc.tensor.matmul(w0_ps[:, :256], ones_row[:, :128],
                                         pt["e2b0"][:], start=False, stop=True)
                    y0 = sb.tile([128, 256], BF16, tag="y0")
                    y0v = y0[:].rearrange("p (i o) -> p i o", o=H)
                    w0v = w0_ps[:, :256].rearrange("p (i o) -> p i o", o=H)
                    xv = xsrc0[:, t, :].unsqueeze(2).broadcast_to((128, NODE_IN, H))
                    nc.vector.tensor_tensor(y0v, w0v, xv, OP.mult)
                    u1 = sb.tile([128, 128], BF16, tag="u1")
                    nc.vector.tensor_tensor(u1[:], y0[:, :128], y0[:, 128:], OP.add)
                    msg = msgp.tile([128, H], F32, tag="msg")
                    nc.vector.tensor_tensor(msg[:], u1[:, :64], u1[:, 64:], OP.add)
                    msg_tiles[t] = msg

                while next_w < NW and 4 * g + gt >= (next_w + 1) * T_w:
                    _emit_window_agg(X, msg_tiles, next_w, layer=0, hr_all=hr0_all)
                    next_w += 1
            _emit_nodes_post(X, 0, hr0_all, h1chunk)

            # ---- AllGather h1, gather h1[src] ----
            if BUILD_STAGE < 2:
                raise _StopBuild()
            nc.gpsimd.collective_compute(
                "AllGather", OP.bypass, replica_groups=RG,
                ins=[h1chunk[:]], outs=[h1full[:]])
            hs_chunks = []
            for b in range(0, E_pad, 1024):
                nn = min(1024, E_pad - b)
                hc = cp.tile([128, nn // 128, H], F32, tag=f"hs1_{b}",
                             name=f"hs1_{b}")
                nc.gpsimd.dma_gather(
                    hc[:], h1full[:],
                    gidx[:, b // 16:(b + nn) // 16], nn, nn, H)
                hs_chunks.append(hc)

            # ================= layer 1 (conv1) =================
            if BUILD_STAGE < 3:
                raise _StopBuild()
            msg_tiles1 = {}
            next_w = 0
            for g in range(NG):
                gt = min(4, NT - 4 * g)
                ncols = gt * 128
                ea_g = sb.tile([4, 512], BF16, tag="ea_g")
                nc.sync.dma_start(ea_g[:, :ncols], din["eaT"][:, g * 512: g * 512 + ncols])
                aT1_ps = psA.tile([128, 512], F32, tag="aT_ps")
                nc.tensor.matmul(aT1_ps[:, :ncols], pt["e1w1"][:], ea_g[:, :ncols],
                                 start=True, stop=not has_e1b1)
                if has_e1b1:
                    nc.tensor.matmul(aT1_ps[:, :ncols], pt["e1b1"][:],
                                     ones_row[:, :ncols], start=False, stop=True)
                aT1 = sb.tile([128, 512], BF16, tag="aT1_sb")
                nc.scalar.activation(aT1[:, :ncols], aT1_ps[:, :ncols], ACTF.Relu)

                for ti in range(gt):
                    t = 4 * g + ti
                    y = sb.tile([128, 4096], BF16, tag="y")
                    for q in range(4):
                        w1q = psW.tile([128, 1024], F32, tag="wq")
                        for hf in range(2):
                            lo = q * 1024 + hf * 512
                            nc.tensor.matmul(
                                w1q[:, hf * 512:(hf + 1) * 512],
                                aT1[:, ti * 128:(ti + 1) * 128],
                                pt["e2w1"][:, lo: lo + 512],
                                start=True, stop=not has_e2b1)
                            if has_e2b1:
                                nc.tensor.matmul(
                                    w1q[:, hf * 512:(hf + 1) * 512],
                                    ones_row[:, :128],
                                    pt["e2b1"][:, lo: lo + 512],
                                    start=False, stop=True)
                        yv = y[:, q * 1024:(q + 1) * 1024].rearrange(
                            "p (i o) -> p i o", o=H)
                        w1v = w1q[:].rearrange("p (i o) -> p i o", o=H)
                        hv = hs_chunks[t // 8][:, t % 8, q * 16:(q + 1) * 16]\
                            .unsqueeze(2).broadcast_to((128, 16, H))
                        nc.vector.tensor_tensor(yv, w1v, hv, OP.mult)
                    # fused bf16 tree over i: 64 -> 32 -> ... -> 1
                    t1 = sb.tile([128, 2048], BF16, tag="tr1")
                    nc.vector.tensor_tensor(t1[:], y[:, :2048], y[:, 2048:], OP.add)
                    t2 = sb.tile([128, 1024], BF16, tag="tr2")
                    nc.vector.tensor_tensor(t2[:], t1[:, :1024], t1[:, 1024:], OP.add)
                    t3 = sb.tile([128, 512], BF16, tag="tr3")
                    nc.vector.tensor_tensor(t3[:], t2[:, :512], t2[:, 512:], OP.add)
                    t4 = sb.tile([128, 256], BF16, tag="tr4")
                    nc.vector.tensor_tensor(t4[:], t3[:, :256], t3[:, 256:], OP.add)
                    t5 = sb.tile([128, 128], BF16, tag="tr5")
                    nc.vector.tensor_tensor(t5[:], t4[:, :128], t4[:, 128:], OP.add)
                    msg = msgp.tile([128, H], F32, tag="msg")
                    nc.vector.tensor_tensor(msg[:], t5[:, :64], t5[:, 64:], OP.add)
                    msg_tiles1[t] = msg

                while next_w < NW and 4 * g + gt >= (next_w + 1) * T_w:
                    _emit_window_agg(X, msg_tiles1, next_w, layer=1, hr_all=hr1_all)
                    next_w += 1
            hw1_all = _emit_nodes_post(X, 1, hr1_all, h2chunk)

            # ---- AllGather h2 + AllReduce g ----
            if BUILD_STAGE < 4:
                raise _StopBuild()
            nc.gpsimd.collective_compute(
                "AllGather", OP.bypass, replica_groups=RG,
                ins=[h2chunk[:]], outs=[h2full[:]])
            ones_col = cp.tile([128, 1], F32, tag="ones_col")
            nc.vector.memset(ones_col[:], 1.0)
            g_ps = psT.tile([128, 128], F32, tag="psT")
            for w in range(NW):
                nreal = min(128, NPC - w * 128)
                nc.tensor.matmul(g_ps[:H, :1], hw1_all[:nreal, w, :],
                                 ones_col[:nreal, :],
                                 start=(w == 0), stop=(w == NW - 1))
            gsum = sb.tile([H, 1], F32, tag="gsum")
            nc.vector.tensor_copy(gsum[:], g_ps[:H, :1])
            nc.sync.dma_start(gin[:], gsum[:])
            nc.gpsimd.collective_compute(
                "AllReduce", OP.add, replica_groups=RG,
                ins=[gin[:]], outs=[gout[:]])

            # ---- global head (redundant on every core) ----
            if BUILD_STAGE < 5:
                raise _StopBuild()
            gcol = sb.tile([H, 1], F32, tag="gcol")
            nc.sync.dma_start(gcol[:], gout[:])
            gcols = sb.tile([H, 1], F32, tag="gcols")
            nc.vector.tensor_scalar_mul(gcols[:], gcol[:], 1.0 / N)
            ghp = psT.tile([128, 128], F32, tag="psT")
            nc.tensor.matmul(ghp[:1, :H], gcols[:], pt["ghw"][:], start=True, stop=False)
            nc.tensor.matmul(ghp[:1, :H], ones_row[:, :1], pt["ghb"][:],
                             start=False, stop=True)
            ghr = sb.tile([1, H], F32, tag="ghr")
            nc.scalar.activation(ghr[:], ghp[:1, :H], ACTF.Relu)
            ghf = sb.tile([1, H], F32, tag="ghf")
            _emit_layernorm(nc, lnp, ghr[:], pt["glng"][:], pt["glnb"][:], ghf[:], 1, H, eps_t)
            qgl = sb.tile([1, 2], F32, tag="qgl")
            junkg = sb.tile([1, H], F32, tag="junkg")
            for j in range(2):
                nc.vector.scalar_tensor_tensor(
                    junkg[:], ghf[:], 1.0, pt["qgw"][:, j * H:(j + 1) * H], OP.mult, OP.mult,
                    accum_out=qgl[:, j:j + 1])
            qglo = sb.tile([1, 2], F32, tag="qglo")
            nc.vector.tensor_tensor(qglo[:], qgl[:], pt["qgb"][:], OP.add)
            nc.sync.dma_start(out_qg[:], qglo[:])
            qadd = sb.tile([1, 2], F32, tag="qadd")
            nc.vector.tensor_tensor(qadd[:], qglo[:], pt["qnb"][:], OP.add)
            qadd_ps = psT.tile([128, 128], F32, tag="psT")
            nc.tensor.matmul(qadd_ps[:, :2], ones_row[:, :128], qadd[:],
                             start=True, stop=True)
            qadd_b = sb.tile([128, 2], F32, tag="qadd_b")
            nc.vector.tensor_copy(qadd_b[:], qadd_ps[:, :2])

            # ---- pipe head ----
            if BUILD_STAGE < 6:
                raise _StopBuild()
            puc, pvc = [], []
            gsu_t = cp.tile([128, PPAD // 16], I16, tag="gsu")
            gsv_t = cp.tile([128, PPAD // 16], I16, tag="gsv")
            nc.sync.dma_start(gsu_t[:], din["gsu"][:])
            nc.sync.dma_start(gsv_t[:], din["gsv"][:])
            for b in range(0, PPAD, 1024):
                nn = min(1024, PPAD - b)
                pc1 = cp.tile([128, nn // 128, H], F32, tag=f"pu_{b}",
                              name=f"pu_{b}")
                nc.gpsimd.dma_gather(
                    pc1[:], h2full[:],
                    gsu_t[:, b // 16:(b + nn) // 16], nn, nn, H)
                puc.append(pc1)
                pc2 = cp.tile([128, nn // 128, H], F32, tag=f"pv_{b}",
                              name=f"pv_{b}")
                nc.gpsimd.dma_gather(
                    pc2[:], h2full[:],
                    gsv_t[:, b // 16:(b + nn) // 16], nn, nn, H)
                pvc.append(pc2)

            m_t = cp.tile([128, PT * 2], F32, tag="m_t")
            cb_t = cp.tile([128, PT * 2], F32, tag="cb_t")
            nc.sync.dma_start(m_t[:], din["m"][:])
            nc.sync.dma_start(cb_t[:], din["cb"][:])

            fpre = cp.tile([128, PT, H], F32, tag="fpre")
            for t in range(PT):
                pairT = sb.tile([128, 128], F32, tag="pairT")
                for half, srcl in ((0, puc), (1, pvc)):
                    srct = srcl[t // 8]
                    tr_ps = psT.tile([128, 128], F32, tag="psT")
                    nc.tensor.transpose(tr_ps[:H, :], srct[:, t % 8, :],
                                        pt["ident"][:])
                    nc.scalar.activation(pairT[half * H:(half + 1) * H, :],
                                         tr_ps[:H, :], ACTF.Copy)
                feat_ps = psT.tile([128, 128], F32, tag="psT")
                nc.tensor.matmul(feat_ps[:, :H], pairT[:], pt["pairw"][:],
                                 start=True, stop=True)
                nc.scalar.activation(fpre[:, t, :], feat_ps[:, :H], ACTF.Copy)
            # batched bias + relu + LN + qn head
            nc.vector.tensor_tensor(
                fpre[:], fpre[:],
                pt["pairb"][:].unsqueeze(1).broadcast_to((128, PT, H)), OP.add)
            nc.scalar.activation(fpre[:], fpre[:], ACTF.Relu)
            featf = cp.tile([128, PT, H], F32, tag="featf")
            _emit_layernorm_b(nc, lnp, fpre[:], pt["plng"], pt["plnb"], featf[:],
                              128, PT, H, eps_t, tagp="p")
            qall = cp.tile([128, PT, 2], F32, tag="qall")
            qtmp = sb.tile([128, PT, H], F32, tag="qtmp")
            for j in range(2):
                nc.vector.tensor_tensor(
                    qtmp[:], featf[:],
                    pt["qnw"][:, j * H:(j + 1) * H].unsqueeze(1).broadcast_to(
                        (128, PT, H)), OP.mult)
                nc.vector.reduce_sum(qall[:, :, j], qtmp[:], axis=AX.X)
            qfin = sb.tile([128, PT * 2], F32, tag="qfin")
            qaddv = qadd_b[:].unsqueeze(1).broadcast_to((128, PT, 2))
            nc.vector.tensor_tensor(qfin[:].rearrange("p (t j) -> p t j", j=2),
                                    qall[:], qaddv, OP.add)
            nc.vector.tensor_tensor(qfin[:], qfin[:], m_t[:], OP.mult)
            nc.vector.tensor_tensor(qfin[:], qfin[:], cb_t[:], OP.add)
            nc.sync.dma_start(out_q[:], qfin[:])
      except _StopBuild:
        pass

    nc.compile()
    return nc


def _emit_window_agg(X, msg_tiles, w, layer, hr_all):
    """Aggregation for one 128-node window -> relu into hr_all[:, w, :]."""
    nc, sb, T_w, pt = X.nc, X.sb, X.T_w, X.pt
    agg = X.psG.tile([128, H], F32, tag="agg")
    if layer == 0:
        nc.tensor.matmul(agg[:], X.xT[:, w * 128:(w + 1) * 128], pt["root0"][:],
                         start=True, stop=False)
        if X.has_bias0:
            nc.tensor.matmul(agg[:], X.ones[:, :128], pt["bias0"][:],
                             start=False, stop=False)
    else:
        nc.tensor.matmul(agg[:], X.h1T_tiles[w][:], pt["root1"][:],
                         start=True, stop=False)
        if X.has_bias1:
            nc.tensor.matmul(agg[:], X.ones[:, :128], pt["bias1"][:],
                             start=False, stop=False)
    for s in range(T_w):
        t = w * T_w + s
        oh = sb.tile([128, 128], F32, tag="oh")
        nc.sync.dma_start(oh[:], X.din["onehot"][t])
        nc.tensor.matmul(agg[:], oh[:], msg_tiles.pop(t)[:],
                         start=False, stop=(s == T_w - 1))
    nc.scalar.activation(hr_all[:, w, :], agg[:], ACTF.Relu)


def _emit_nodes_post(X, layer, hr_all, chunk):
    """Batched LN over all windows, DMA chunks, per-layer extras."""
    nc = X.nc
    hw_all = X.cp.tile([128, NW, H], F32, tag=f"hwall{layer}",
                       name=f"hwall{layer}")
    g_b = X.pt["ln0g"] if layer == 0 else X.pt["ln1g"]
    b_b = X.pt["ln0b"] if layer == 0 else X.pt["ln1b"]
    _emit_layernorm_b(nc, X.lnp, hr_all[:], g_b, b_b, hw_all[:], 128, NW, H,
                      X.eps, tagp=str(layer))
    for w in range(NW):
        nc.sync.dma_start(chunk[w * 128:(w + 1) * 128, :], hw_all[:, w, :])
        if layer == 0:
            tr_ps = X.psT.tile([128, 128], F32, tag="psT", name=f"trp{w}")
            nc.tensor.transpose(tr_ps[:H, :], hw_all[:, w, :], X.pt["ident"][:])
            hT = X.hold.tile([H, 128], F32, tag="hT", name=f"hT{w}")
            nc.vector.tensor_copy(hT[:], tr_ps[:H, :])
            X.h1T_tiles.append(hT)
    return hw_all


def kernel(x, edge_index, edge_attr, pipe_edge_idx, pipe_open_mask, params):
    x = np.asarray(x, np.float32)
    edge_index = np.asarray(edge_index)
    edge_attr = np.asarray(edge_attr, np.float32)
    pipe_edge_idx = np.asarray(pipe_edge_idx)
    pipe_open_mask = np.asarray(pipe_open_mask, np.float32)

    cores, T_w, NT, E_pad = _host_prep(x, edge_index, edge_attr,
                                       pipe_edge_idx, pipe_open_mask)
    pp = _prep_params(params)

    nc = build_program(T_w, NT, E_pad, pp)

    import ml_dtypes
    in_maps = []
    for c in range(NCORES):
        m = dict(pp)
        m.update(cores[c])
        mm = {}
        for k, v in m.items():
            if k in ("e2w0", "e2w1", "e1w0", "e1w1", "e1b0", "e1b1", "eaT"):
                v = np.asarray(v).astype(ml_dtypes.bfloat16)
            mm[k] = np.ascontiguousarray(v)
        in_maps.append(mm)

    res = run_bass_kernel_spmd(nc, in_maps, core_ids=list(range(NCORES)))
    global LAST_RESULT
    LAST_RESULT = res

    qs = []
    for c in range(NCORES):
        oq = np.asarray(res.results[c]["out_q"]).reshape(128, PT, 2)
        qs.append(oq.transpose(1, 0, 2).reshape(PPAD, 2)[:PPC])
    q = np.concatenate(qs, axis=0)
    qg = np.asarray(res.results[0]["out_qg"]).reshape(1, 2)
    out = np.concatenate([q.reshape(1, -1), qg], axis=1).astype(np.float32)
    return out
